# revision 34
# baseline (speedup 1.0000x reference)
"""PointsRenderer (alpha compositing over K points/pixel) on 8 trn2 cores.

Sharding: data-parallel over batch B=8 -> 1 image per NeuronCore.

Gather strategy: the [100000, 4] feature table is repacked host-side into
256B-pitch blocks of 4 consecutive rows (feat256[q, 0:16] = rows 4q..4q+3,
rest pad).  Each fragment's feature row is fetched with batched
InstDMAGatherAnt calls: 8192 int16 block indices (idx>>2, all < 25000)
per call gather 64B each; the required 16B sub-row is then selected on
the vector engine with a 4-way residue (idx&3) mask fused with the
compositing weight via scalar_tensor_tensor.

Performance structure (measured on HW):
- The gather ucode (dma_gather.cpp) runs on the Q7 core pair selected by
  queue_num (cpu_id/2 == queue_num); cores walk the instruction stream
  independently, so round-robining queue_num over all 4 SWDGE queues runs
  descriptor generation on all 8 Q7 cores concurrently (~2.6ns/desc
  aggregate, the platform floor; bytes/descriptor don't matter below the
  ~7ns min-transfer floor, so fp32 64B blocks are free vs bf16).
- Compositing is hoisted to chunk level (CH=8 tiles = 512 frag/partition
  per DVE op) to keep the DVE instruction count (~1k total) off the
  critical path; per-tile DVE issue overhead (~1us/instr) otherwise
  dominates.
- 21.4ms (baseline per-partition indirect1d) -> ~4.66ms/core, at the Q7
  descriptor-generation floor: 2M descriptors/core x ~2.6ns. bf16 table
  halves the G tile so 6 chunk buffers (48 outstanding gathers) fit in
  SBUF - that buffering depth, not the byte count, is what bf16 buys
  (rel err 3.1e-03 vs the 2e-2 gate). gp bufs=8 regresses (SBUF pressure).

Compositing math per pixel (K=8 fragments, front-to-back):
alpha_k = 1 - d2*inv_r2, contrib_k = alpha_k * prod_{j<k}(d2_j*inv_r2),
out = sum_k contrib_k * feat[idx_k] (tree reduction over K).
"""

import numpy as np

import concourse.bass as bass
import concourse.mybir as mybir
import concourse.tile as tile
from concourse import bacc, library_config
from concourse.bass_utils import run_bass_kernel_spmd

B, H, W, K, P, C = 8, 512, 512, 8, 100000, 4
PARTS = 128
NF = H * W * K            # fragments per core (B=1 shard)
PERPART = NF // PARTS     # 16384
NIDX = 8192               # indices per gather call (HW cap is <16384)
FT = NIDX // PARTS        # 64 fragments per partition per tile
PIX_T = FT // K           # 8 pixels per partition per tile
NT = PERPART // FT        # 256 tiles
QROWS = P // 4            # 25000 packed blocks
QPITCH = 128              # bf16 per block row (256B pitch)
QELEM = 16                # bf16 gathered per index (32B = one 4-row block)

F32 = mybir.dt.float32
BF16 = mybir.dt.bfloat16
I16 = mybir.dt.int16


def emit_dma_gather16(nc, out_ap, in_ap, idxs_ap, num_idxs, elem_size,
                      elem_step, queue_num=0):
    """InstDMAGatherAnt with elem_size*dtype not a multiple of 256B.

    Same lowering as BassGpSimd.dma_gather(transpose=False) minus its
    256B elem-size assert; the ISA only needs the row PITCH (elem_step)
    to be a 256B multiple (stride_bytes_256 field).
    """
    gpsimd = nc.gpsimd
    assert in_ap.ap[0][0] == elem_step
    stride_bytes = elem_step * mybir.dt.size(in_ap.dtype)
    stride_bytes_256 = stride_bytes // 256
    assert stride_bytes % 256 == 0 and 0 < stride_bytes_256 < 256
    _in_ap = gpsimd.lower_ap_dma(in_ap, for_custom_bir_dma=True)
    _idxs_ap = gpsimd.lower_ap(idxs_ap)
    _out_ap = gpsimd.lower_ap(out_ap)
    return gpsimd.add_instruction(
        mybir.InstDMAGatherAnt(
            name=nc.get_next_instruction_name(),
            ins=[*_in_ap, _idxs_ap,
                 gpsimd.lower_val_access(gpsimd.to_reg(num_idxs))],
            outs=[_out_ap],
            transpose=False,
            num_idxs=num_idxs,
            elem_size=elem_size,
            stride_bytes_256=stride_bytes_256,
            gen_mode=0,
            single_packet=False,
            queue_num=queue_num,
            sbuf_tokens_per_rank=0,
            sbuf_free_dim_per_rank=0,
            sbuf_free_dim_pad_per_rank=0,
            sbuf_byte_offset=0,
        )
    )


NQ = 4                    # SWDGE queues (ucode max 4) - gathers round-robin
CH = 8                    # tiles per chunk (compositing runs at chunk level)
NCHUNK = NT // CH
ITC = NIDX // 16          # idx cols per tile in the wrapped layout
CF = CH * FT              # fragments per partition per chunk (512)
CPIX = CF // K            # pixels per partition per chunk (64)


def build(inv_r2: float, reps: int = 1, nq: int = NQ, ablate: str = "none"):
    """reps>1 repeats the whole per-core workload (for timing: the extra
    reps isolate HW exec time from the ~70ms dispatch floor).
    ablate: 'nogather' drops the dma_gather calls (output garbage),
    'nodve' drops compositing (output garbage) - for bottleneck isolation."""
    nc = bacc.Bacc(None, target_bir_lowering=False, debug=False,
                   num_swdge_queues=nq)
    idx16d = nc.dram_tensor("idx16d", [NCHUNK, 16, CH * ITC], I16,
                            kind="ExternalInput")
    resd = nc.dram_tensor("resd", [PARTS, NT, FT], F32, kind="ExternalInput")
    d2d = nc.dram_tensor("d2d", [PARTS, NT, FT], F32, kind="ExternalInput")
    feat = nc.dram_tensor("feat", [QROWS, QPITCH], BF16, kind="ExternalInput")
    out = nc.dram_tensor("out", [PARTS, NT, PIX_T * C], F32,
                         kind="ExternalOutput")

    EQ = mybir.AluOpType.is_equal
    MUL = mybir.AluOpType.mult

    with tile.TileContext(nc) as tc:
        nc.gpsimd.load_library(library_config.mlp)
        with tc.tile_pool(name="io", bufs=4) as io, \
             tc.tile_pool(name="gp", bufs=6) as gp, \
             tc.tile_pool(name="wp", bufs=2) as wp, \
             tc.tile_pool(name="op", bufs=2) as op:
            for rep in range(reps):
                for ch in range(NCHUNK):
                    t0 = ch * CH
                    # idx16 chunk: replicate the wrapped [16, CH*512] block
                    # into all 8 gpsimd 16-partition groups with independent
                    # parallel DRAM loads (no serial doubling-chain latency)
                    it = io.tile([PARTS, CH * ITC], I16, tag="it")
                    for g in range(8):
                        nc.sync.dma_start(it[g * 16:(g + 1) * 16, :],
                                          idx16d[ch])

                    d2c = io.tile([PARTS, CF], F32, tag="d2c")
                    nc.sync.dma_start(
                        d2c[:], d2d[:, t0:t0 + CH, :].rearrange("p t f -> p (t f)"))
                    resc = io.tile([PARTS, CF], F32, tag="resc")
                    nc.sync.dma_start(
                        resc[:], resd[:, t0:t0 + CH, :].rearrange("p t f -> p (t f)"))

                    outc = op.tile([PARTS, CH, PIX_T, C], F32, tag="outc")

                    # CH batched gathers into slices of one chunk G tile
                    G = gp.tile([PARTS, CH, FT, QELEM], BF16, tag="G")
                    if ablate != "nogather":
                        for j in range(CH):
                            emit_dma_gather16(
                                nc, G[:, j, :, :], feat[:, 0:QELEM],
                                it[:, j * ITC:(j + 1) * ITC],
                                num_idxs=NIDX, elem_size=QELEM,
                                elem_step=QPITCH,
                                queue_num=(t0 + j) % nq,
                            )
                    if ablate == "nodve":
                        nc.vector.tensor_copy(
                            outc[:].rearrange("p t x c -> p (t x) c"),
                            G[:, :, 0:PIX_T, 0:C].rearrange("p t x c -> p (t x) c"),
                        )
                        nc.sync.dma_start(
                            out[:, t0:t0 + CH, :],
                            outc[:].rearrange("p t x c -> p t (x c)"),
                        )
                        continue

                    # chunk-level compositing on [128, CF]
                    # alpha_k = 1 - d2*inv_r2 (ACT), om_k = d2*inv_r2 (DVE)
                    alpha = wp.tile([PARTS, CF], F32, tag="alpha")
                    nc.scalar.activation(
                        alpha[:], d2c[:], mybir.ActivationFunctionType.Copy,
                        bias=1.0, scale=-float(inv_r2),
                    )
                    om = wp.tile([PARTS, CF], F32, tag="om")
                    nc.vector.tensor_scalar_mul(om[:], d2c[:], float(inv_r2))

                    # contrib_k = alpha_k * prod_{j<k} om_j over [128, CPIX, K]
                    cb = wp.tile([PARTS, CF], F32, tag="cb")
                    cbv = cb[:].rearrange("p (x k) -> p x k", k=K)
                    alv = alpha[:].rearrange("p (x k) -> p x k", k=K)
                    omv = om[:].rearrange("p (x k) -> p x k", k=K)
                    rt = wp.tile([PARTS, CPIX], F32, tag="rt")
                    nc.vector.tensor_copy(cbv[:, :, 0], alv[:, :, 0])
                    nc.vector.tensor_copy(rt[:], omv[:, :, 0])
                    for k in range(1, K):
                        nc.vector.tensor_mul(cbv[:, :, k], alv[:, :, k], rt[:])
                        if k < K - 1:
                            nc.vector.tensor_mul(rt[:], rt[:], omv[:, :, k])

                    # acc = sum_r (res==r)*contrib * G[..., 4r:4r+4]
                    Gf = G[:].rearrange("p t f g -> p (t f) g")
                    acc = wp.tile([PARTS, CF, C], F32, tag="acc")
                    tmp = wp.tile([PARTS, CF, C], F32, tag="tmp")
                    cmr = wp.tile([PARTS, CF], F32, tag="cmr")
                    for r in range(4):
                        nc.vector.scalar_tensor_tensor(
                            cmr[:], resc[:], float(r), cb[:], EQ, MUL,
                        )
                        cmb = cmr[:].rearrange(
                            "p (f one) -> p f one", one=1
                        ).to_broadcast([PARTS, CF, C])
                        dst = acc[:] if r == 0 else tmp[:]
                        nc.vector.tensor_mul(dst, Gf[:, :, 4 * r:4 * r + 4], cmb)
                        if r > 0:
                            nc.vector.tensor_add(acc[:], acc[:], tmp[:])

                    # sum over K: tree reduction into the chunk out tile
                    av = acc[:].rearrange("p (x k) c -> p x k c", k=K)
                    nc.vector.tensor_add(av[:, :, 0:4, :], av[:, :, 0:4, :],
                                         av[:, :, 4:8, :])
                    nc.vector.tensor_add(av[:, :, 0:2, :], av[:, :, 0:2, :],
                                         av[:, :, 2:4, :])
                    nc.vector.tensor_add(
                        outc[:].rearrange("p t x c -> p (t x) c"),
                        av[:, :, 0, :], av[:, :, 1, :])

                    nc.sync.dma_start(
                        out[:, t0:t0 + CH, :],
                        outc[:].rearrange("p t x c -> p t (x c)"),
                    )

    nc.compile()
    return nc


def prepare_inputs(idx, dists2, features):
    """Host-side repacking -> per-core in_maps (shared feat table)."""
    idx = np.ascontiguousarray(idx)
    dists2 = np.ascontiguousarray(dists2, dtype=np.float32)
    features = np.ascontiguousarray(features, dtype=np.float32)

    import ml_dtypes
    feat256 = np.zeros((QROWS, QPITCH), ml_dtypes.bfloat16)
    feat256[:, 0:QELEM] = features.reshape(QROWS, QELEM).astype(
        ml_dtypes.bfloat16)

    in_maps = []
    for b in range(B):
        idxf = idx[b].reshape(NF)
        q = (idxf >> 2).astype(np.int16).reshape(PARTS, NT, FT)
        res = (idxf & 3).astype(np.float32).reshape(PARTS, NT, FT)
        d2b = dists2[b].reshape(PARTS, NT, FT)
        # stream position i = f*128 + p; wrapped[r, c] = stream[c*16 + r];
        # chunk layout: [NCHUNK, 16, CH*ITC], col block j holds tile t0+j
        stream = q.transpose(1, 2, 0).reshape(NT, NIDX)
        wrapped = stream.reshape(NT, ITC, 16).transpose(0, 2, 1)  # [NT,16,ITC]
        idx16d = np.ascontiguousarray(
            wrapped.reshape(NCHUNK, CH, 16, ITC).transpose(0, 2, 1, 3)
            .reshape(NCHUNK, 16, CH * ITC)
        )
        in_maps.append({
            "idx16d": idx16d,
            "resd": np.ascontiguousarray(res),
            "d2d": np.ascontiguousarray(d2b),
            "feat": feat256,
        })
    return in_maps


def assemble(results):
    out = np.empty((B, H, W, C), dtype=np.float32)
    for b in range(B):
        # out dram [128, NT, PIX_T*C]: pixel flat = p*2048 + t*8 + pix
        out[b] = results[b]["out"].reshape(H * W, C).reshape(H, W, C)
    return out


def kernel(idx, dists2, features, radius):
    r = float(np.asarray(radius).reshape(-1)[0])
    inv_r2 = 1.0 / (r * r)
    nc = build(inv_r2)
    in_maps = prepare_inputs(idx, dists2, features)
    res = run_bass_kernel_spmd(nc, in_maps, core_ids=list(range(B)))
    return assemble(res.results)


# revision 36
# speedup vs baseline: 1.0204x; 1.0204x over previous
"""PointsRenderer (alpha compositing over K points/pixel) on 8 trn2 cores.

Sharding: data-parallel over batch B=8 -> 1 image per NeuronCore.

Gather strategy: the [100000, 4] feature table is repacked host-side into
256B-pitch blocks of 4 consecutive rows (feat256[q, 0:16] = rows 4q..4q+3,
rest pad).  Each fragment's feature row is fetched with batched
InstDMAGatherAnt calls: 8192 int16 block indices (idx>>2, all < 25000)
per call gather 64B each; the required 16B sub-row is then selected on
the vector engine with a 4-way residue (idx&3) mask fused with the
compositing weight via scalar_tensor_tensor.

Performance structure (measured on HW):
- The gather ucode (dma_gather.cpp) runs on the Q7 core pair selected by
  queue_num (cpu_id/2 == queue_num); cores walk the instruction stream
  independently, so round-robining queue_num over all 4 SWDGE queues runs
  descriptor generation on all 8 Q7 cores concurrently (~2.6ns/desc
  aggregate, the platform floor; bytes/descriptor don't matter below the
  ~7ns min-transfer floor, so fp32 64B blocks are free vs bf16).
- Compositing is hoisted to chunk level (CH=8 tiles = 512 frag/partition
  per DVE op) to keep the DVE instruction count (~1k total) off the
  critical path; per-tile DVE issue overhead (~1us/instr) otherwise
  dominates.
- 21.4ms (baseline per-partition indirect1d) -> ~4.66ms/core, at the Q7
  descriptor-generation floor: 2M descriptors/core x ~2.6ns. bf16 table
  halves the G tile so 6 chunk buffers (48 outstanding gathers) fit in
  SBUF - that buffering depth, not the byte count, is what bf16 buys
  (rel err 3.1e-03 vs the 2e-2 gate). gp bufs=8 regresses (SBUF pressure).

Compositing math per pixel (K=8 fragments, front-to-back):
alpha_k = 1 - d2*inv_r2, contrib_k = alpha_k * prod_{j<k}(d2_j*inv_r2),
out = sum_k contrib_k * feat[idx_k] (tree reduction over K).
"""

import numpy as np

import concourse.bass as bass
import concourse.mybir as mybir
import concourse.tile as tile
from concourse import bacc, library_config
from concourse.bass_utils import run_bass_kernel_spmd

B, H, W, K, P, C = 8, 512, 512, 8, 100000, 4
PARTS = 128
NF = H * W * K            # fragments per core (B=1 shard)
PERPART = NF // PARTS     # 16384
NIDX = 8192               # indices per gather call (HW cap is <16384)
FT = NIDX // PARTS        # 64 fragments per partition per tile
PIX_T = FT // K           # 8 pixels per partition per tile
NT = PERPART // FT        # 256 tiles
QROWS = P // 4            # 25000 packed blocks
QPITCH = 128              # bf16 per block row (256B pitch)
QELEM = 16                # bf16 gathered per index (32B = one 4-row block)

F32 = mybir.dt.float32
BF16 = mybir.dt.bfloat16
I16 = mybir.dt.int16


def emit_dma_gather16(nc, out_ap, in_ap, idxs_ap, num_idxs, elem_size,
                      elem_step, queue_num=0):
    """InstDMAGatherAnt with elem_size*dtype not a multiple of 256B.

    Same lowering as BassGpSimd.dma_gather(transpose=False) minus its
    256B elem-size assert; the ISA only needs the row PITCH (elem_step)
    to be a 256B multiple (stride_bytes_256 field).
    """
    gpsimd = nc.gpsimd
    assert in_ap.ap[0][0] == elem_step
    stride_bytes = elem_step * mybir.dt.size(in_ap.dtype)
    stride_bytes_256 = stride_bytes // 256
    assert stride_bytes % 256 == 0 and 0 < stride_bytes_256 < 256
    _in_ap = gpsimd.lower_ap_dma(in_ap, for_custom_bir_dma=True)
    _idxs_ap = gpsimd.lower_ap(idxs_ap)
    _out_ap = gpsimd.lower_ap(out_ap)
    return gpsimd.add_instruction(
        mybir.InstDMAGatherAnt(
            name=nc.get_next_instruction_name(),
            ins=[*_in_ap, _idxs_ap,
                 gpsimd.lower_val_access(gpsimd.to_reg(num_idxs))],
            outs=[_out_ap],
            transpose=False,
            num_idxs=num_idxs,
            elem_size=elem_size,
            stride_bytes_256=stride_bytes_256,
            gen_mode=0,
            single_packet=False,
            queue_num=queue_num,
            sbuf_tokens_per_rank=0,
            sbuf_free_dim_per_rank=0,
            sbuf_free_dim_pad_per_rank=0,
            sbuf_byte_offset=0,
        )
    )


NQ = 4                    # SWDGE queues (ucode max 4) - gathers round-robin
CH = 8                    # tiles per chunk (compositing runs at chunk level)
NCHUNK = NT // CH
ITC = NIDX // 16          # idx cols per tile in the wrapped layout
CF = CH * FT              # fragments per partition per chunk (512)
CPIX = CF // K            # pixels per partition per chunk (64)


def build(inv_r2: float, reps: int = 1, nq: int = NQ, ablate: str = "none"):
    """reps>1 repeats the whole per-core workload (for timing: the extra
    reps isolate HW exec time from the ~70ms dispatch floor).
    ablate: 'nogather' drops the dma_gather calls (output garbage),
    'nodve' drops compositing (output garbage) - for bottleneck isolation."""
    nc = bacc.Bacc(None, target_bir_lowering=False, debug=False,
                   num_swdge_queues=nq)
    idx16d = nc.dram_tensor("idx16d", [NCHUNK, 16, CH * ITC], I16,
                            kind="ExternalInput")
    resd = nc.dram_tensor("resd", [PARTS, NT, FT], F32, kind="ExternalInput")
    d2d = nc.dram_tensor("d2d", [PARTS, NT, FT], F32, kind="ExternalInput")
    feat = nc.dram_tensor("feat", [QROWS, QPITCH], BF16, kind="ExternalInput")
    out = nc.dram_tensor("out", [PARTS, NT, PIX_T * C], F32,
                         kind="ExternalOutput")

    EQ = mybir.AluOpType.is_equal
    MUL = mybir.AluOpType.mult

    with tile.TileContext(nc) as tc:
        nc.gpsimd.load_library(library_config.mlp)
        with tc.tile_pool(name="io", bufs=2) as io, \
             tc.tile_pool(name="ip", bufs=3) as ip, \
             tc.tile_pool(name="gp", bufs=6) as gp, \
             tc.tile_pool(name="wp", bufs=2) as wp, \
             tc.tile_pool(name="op", bufs=2) as op:
            for rep in range(reps):
                for ch in range(NCHUNK):
                    t0 = ch * CH
                    # idx16 chunk: replicate the wrapped [16, CH*512] block
                    # into all 8 gpsimd 16-partition groups with independent
                    # parallel DRAM loads (no serial doubling-chain latency)
                    it = ip.tile([PARTS, CH * ITC], I16, tag="it")
                    for g in range(8):
                        nc.sync.dma_start(it[g * 16:(g + 1) * 16, :],
                                          idx16d[ch])

                    d2c = io.tile([PARTS, CF], F32, tag="d2c")
                    nc.sync.dma_start(
                        d2c[:], d2d[:, t0:t0 + CH, :].rearrange("p t f -> p (t f)"))
                    resc = io.tile([PARTS, CF], F32, tag="resc")
                    nc.sync.dma_start(
                        resc[:], resd[:, t0:t0 + CH, :].rearrange("p t f -> p (t f)"))

                    outc = op.tile([PARTS, CH, PIX_T, C], F32, tag="outc")

                    # CH batched gathers into slices of one chunk G tile
                    G = gp.tile([PARTS, CH, FT, QELEM], BF16, tag="G")
                    if ablate != "nogather":
                        for j in range(CH):
                            emit_dma_gather16(
                                nc, G[:, j, :, :], feat[:, 0:QELEM],
                                it[:, j * ITC:(j + 1) * ITC],
                                num_idxs=NIDX, elem_size=QELEM,
                                elem_step=QPITCH,
                                queue_num=(t0 + j) % nq,
                            )
                    if ablate == "nodve":
                        nc.vector.tensor_copy(
                            outc[:].rearrange("p t x c -> p (t x) c"),
                            G[:, :, 0:PIX_T, 0:C].rearrange("p t x c -> p (t x) c"),
                        )
                        nc.sync.dma_start(
                            out[:, t0:t0 + CH, :],
                            outc[:].rearrange("p t x c -> p t (x c)"),
                        )
                        continue

                    # chunk-level compositing on [128, CF]
                    # alpha_k = 1 - d2*inv_r2 (ACT), om_k = d2*inv_r2 (DVE)
                    alpha = wp.tile([PARTS, CF], F32, tag="alpha")
                    nc.scalar.activation(
                        alpha[:], d2c[:], mybir.ActivationFunctionType.Copy,
                        bias=1.0, scale=-float(inv_r2),
                    )
                    om = wp.tile([PARTS, CF], F32, tag="om")
                    nc.vector.tensor_scalar_mul(om[:], d2c[:], float(inv_r2))

                    # contrib_k = alpha_k * prod_{j<k} om_j over [128, CPIX, K]
                    cb = wp.tile([PARTS, CF], F32, tag="cb")
                    cbv = cb[:].rearrange("p (x k) -> p x k", k=K)
                    alv = alpha[:].rearrange("p (x k) -> p x k", k=K)
                    omv = om[:].rearrange("p (x k) -> p x k", k=K)
                    rt = wp.tile([PARTS, CPIX], F32, tag="rt")
                    nc.vector.tensor_copy(cbv[:, :, 0], alv[:, :, 0])
                    nc.vector.tensor_copy(rt[:], omv[:, :, 0])
                    for k in range(1, K):
                        nc.vector.tensor_mul(cbv[:, :, k], alv[:, :, k], rt[:])
                        if k < K - 1:
                            nc.vector.tensor_mul(rt[:], rt[:], omv[:, :, k])

                    # acc = sum_r (res==r)*contrib * G[..., 4r:4r+4]
                    Gf = G[:].rearrange("p t f g -> p (t f) g")
                    acc = wp.tile([PARTS, CF, C], F32, tag="acc")
                    tmp = wp.tile([PARTS, CF, C], F32, tag="tmp")
                    cmr = wp.tile([PARTS, CF], F32, tag="cmr")
                    for r in range(4):
                        nc.vector.scalar_tensor_tensor(
                            cmr[:], resc[:], float(r), cb[:], EQ, MUL,
                        )
                        cmb = cmr[:].rearrange(
                            "p (f one) -> p f one", one=1
                        ).to_broadcast([PARTS, CF, C])
                        dst = acc[:] if r == 0 else tmp[:]
                        nc.vector.tensor_mul(dst, Gf[:, :, 4 * r:4 * r + 4], cmb)
                        if r > 0:
                            nc.vector.tensor_add(acc[:], acc[:], tmp[:])

                    # sum over K: tree reduction into the chunk out tile
                    av = acc[:].rearrange("p (x k) c -> p x k c", k=K)
                    nc.vector.tensor_add(av[:, :, 0:4, :], av[:, :, 0:4, :],
                                         av[:, :, 4:8, :])
                    nc.vector.tensor_add(av[:, :, 0:2, :], av[:, :, 0:2, :],
                                         av[:, :, 2:4, :])
                    nc.vector.tensor_add(
                        outc[:].rearrange("p t x c -> p (t x) c"),
                        av[:, :, 0, :], av[:, :, 1, :])

                    nc.sync.dma_start(
                        out[:, t0:t0 + CH, :],
                        outc[:].rearrange("p t x c -> p t (x c)"),
                    )

    nc.compile()
    return nc


def prepare_inputs(idx, dists2, features):
    """Host-side repacking -> per-core in_maps (shared feat table)."""
    idx = np.ascontiguousarray(idx)
    dists2 = np.ascontiguousarray(dists2, dtype=np.float32)
    features = np.ascontiguousarray(features, dtype=np.float32)

    import ml_dtypes
    feat256 = np.zeros((QROWS, QPITCH), ml_dtypes.bfloat16)
    feat256[:, 0:QELEM] = features.reshape(QROWS, QELEM).astype(
        ml_dtypes.bfloat16)

    in_maps = []
    for b in range(B):
        idxf = idx[b].reshape(NF)
        q = (idxf >> 2).astype(np.int16).reshape(PARTS, NT, FT)
        res = (idxf & 3).astype(np.float32).reshape(PARTS, NT, FT)
        d2b = dists2[b].reshape(PARTS, NT, FT)
        # stream position i = f*128 + p; wrapped[r, c] = stream[c*16 + r];
        # chunk layout: [NCHUNK, 16, CH*ITC], col block j holds tile t0+j
        stream = q.transpose(1, 2, 0).reshape(NT, NIDX)
        wrapped = stream.reshape(NT, ITC, 16).transpose(0, 2, 1)  # [NT,16,ITC]
        idx16d = np.ascontiguousarray(
            wrapped.reshape(NCHUNK, CH, 16, ITC).transpose(0, 2, 1, 3)
            .reshape(NCHUNK, 16, CH * ITC)
        )
        in_maps.append({
            "idx16d": idx16d,
            "resd": np.ascontiguousarray(res),
            "d2d": np.ascontiguousarray(d2b),
            "feat": feat256,
        })
    return in_maps


def assemble(results):
    out = np.empty((B, H, W, C), dtype=np.float32)
    for b in range(B):
        # out dram [128, NT, PIX_T*C]: pixel flat = p*2048 + t*8 + pix
        out[b] = results[b]["out"].reshape(H * W, C).reshape(H, W, C)
    return out


def kernel(idx, dists2, features, radius):
    r = float(np.asarray(radius).reshape(-1)[0])
    inv_r2 = 1.0 / (r * r)
    nc = build(inv_r2)
    in_maps = prepare_inputs(idx, dists2, features)
    res = run_bass_kernel_spmd(nc, in_maps, core_ids=list(range(B)))
    return assemble(res.results)
